# revision 1
# baseline (speedup 1.0000x reference)
"""GNN mean-aggregation conv kernel for Trainium2, 8-core SPMD.

Computes out[v] = (1/deg[v]) * sum_{(s,v) in E} (x[s] @ W.T + b), deg by dst.

Strategy: shard destination nodes across 8 cores (12500 rows each).  Use the
linearity of the op to aggregate raw x first and apply the 128x128 linear
second: out = (D^-1 A x) W^T + b*mask.  Edges are grouped by 128-dst block on
the host; each core gathers x[src] rows with dma_gather (int16 indices into
four overlapping 32768-row source windows), segment-sums them with one-hot
matmuls on the PE (aggT[f,d] += G[e,f]^T onehot[e,d]), then applies W^T, a
rank-1 deg*b term and a per-partition 1/deg scale:
out[d,j] = (sum_f aggT[f,d] Wt[f,j] + deg[d] b[j]) * inv_deg[d].
"""

import numpy as np

N, E, D = 100000, 640000, 128
NCORES = 8
NPC = N // NCORES            # dst nodes per core
P = 128                      # partition dim / dst block size
NB = (NPC + P - 1) // P      # 98 dst blocks per core
NPAD = NB * P                # 12544 padded dst rows per core
GROUP = 8                    # dst blocks per gather group
WIN = 32768                  # int16-addressable window
WBASE = [0, 22411, 44822, 67232]
NW = 4


def _build_schedule(edge_index):
    """Host-side prep.

    Returns (sched, per_core) where sched holds the shared tile structure
    (T[b][w] tile counts) and per_core the packed idx/dstl/deg arrays.
    """
    src = np.asarray(edge_index[0], dtype=np.int64)
    dst = np.asarray(edge_index[1], dtype=np.int64)

    deg = np.bincount(dst, minlength=N).astype(np.float32)
    inv_deg = np.where(deg > 0, 1.0 / np.maximum(deg, 1), 0.0).astype(np.float32)

    core = dst // NPC
    local = dst - core * NPC
    blk = local // P
    dstl = (local - blk * P).astype(np.float32)

    # sort edges by (core, block, src)
    key = (core * NB + blk) * (N + 1) + src
    order = np.argsort(key, kind="stable")
    src_s = src[order]
    gblk_s = (core * NB + blk)[order]
    dstl_s = dstl[order]

    starts = np.searchsorted(gblk_s, np.arange(NCORES * NB + 1) - 0.5)

    # per (core, block): edge src arrays (sorted)
    def block_srcs(c, b):
        g = c * NB + b
        return src_s[starts[g] : starts[g + 1]], dstl_s[starts[g] : starts[g + 1]]

    # --- shared per-block window tile counts T[b][w] ---
    T = np.zeros((NB, NW), dtype=np.int64)
    for b in range(NB):
        # forward cumulative: edges that must be in windows <= w
        F = np.zeros(NW, dtype=np.int64)
        maxtot = 0
        for w in range(NW):
            hi = WBASE[w + 1] if w + 1 < NW else N
            m = 0
            for c in range(NCORES):
                s, _ = block_srcs(c, b)
                m = max(m, int(np.searchsorted(s, hi)))
            F[w] = (m + P - 1) // P
        for c in range(NCORES):
            s, _ = block_srcs(c, b)
            maxtot = max(maxtot, len(s))
        F[NW - 1] = max(F[NW - 1], (maxtot + P - 1) // P, 1)
        for w in range(1, NW):
            F[w] = max(F[w], F[w - 1])
        Tb = np.diff(np.concatenate([[0], F]))
        # backward: edges with src >= WBASE[w] must fit in suffix
        for w in range(NW - 1, 0, -1):
            m = 0
            for c in range(NCORES):
                s, _ = block_srcs(c, b)
                m = max(m, len(s) - int(np.searchsorted(s, WBASE[w])))
            need = (m + P - 1) // P
            while Tb[w:].sum() < need:
                Tb[w] += 1
        T[b] = Tb

    # --- per-core greedy assignment + packing, with retry on infeasibility ---
    for _attempt in range(20):
        ok, per_core = _try_pack(T, block_srcs, deg, inv_deg)
        if ok:
            break
        # _try_pack bumped T in place on failure
    else:
        raise RuntimeError("window assignment failed to converge")

    col_off = np.zeros(NB + 1, dtype=np.int64)  # global tile offset per block
    # global tile order: groups of GROUP blocks; within group: w-major, then b
    tile_cols = {}  # (b, w) -> first global tile col
    tcol = 0
    b0 = 0
    while b0 < NB:
        blocks = list(range(b0, min(b0 + GROUP, NB)))
        for w in range(NW):
            for b in blocks:
                tile_cols[(b, w)] = tcol
                tcol += int(T[b, w])
        b0 += GROUP
    Ttot = tcol

    sched = {"T": T, "tile_cols": tile_cols, "Ttot": Ttot}
    # repack per-core arrays into the global layout
    packed = [_pack_core(T, tile_cols, Ttot, pc) for pc in per_core]
    return sched, packed


def _try_pack(T, block_srcs, deg, inv_deg):
    """Greedy per-core window assignment. Returns (ok, per_core_raw).
    On infeasibility bumps T in place and returns (False, None)."""
    per_core = []
    for c in range(NCORES):
        core_asn = {}  # (b, w) -> (idx_list, dstl_list)
        for b in range(T.shape[0]):
            s, dl = block_srcs(c, b)
            n = len(s)
            used = np.zeros(n, dtype=bool)
            for w in range(NW):
                lo = WBASE[w]
                hi = lo + WIN
                cap = int(T[b, w]) * P
                # must-take: not yet used, src in window, and not eligible later
                nxt = WBASE[w + 1] if w + 1 < NW else N
                elig = (~used) & (s >= lo) & (s < hi)
                must = elig & (s < nxt)
                i_must = np.where(must)[0]
                if len(i_must) > cap:
                    T[b, w] += 1
                    return False, None
                take = list(i_must)
                i_opt = np.where(elig & ~must)[0]
                room = cap - len(take)
                take += list(i_opt[:room])
                used[take] = True
                core_asn[(b, w)] = (
                    (s[take] - lo).astype(np.int16),
                    dl[take].astype(np.float32),
                )
            if not used.all():
                T[b, NW - 1] += 1
                return False, None
        per_core.append({"asn": core_asn, "core": c})
    # attach deg data
    for c in range(NCORES):
        base = c * NPC
        tmp = np.zeros(NPAD, dtype=np.float32)
        tmp[:NPC] = inv_deg[base : base + NPC]
        per_core[c]["invdeg"] = np.ascontiguousarray(tmp.reshape(NB, P).T)
        degr = np.zeros((1, NPAD), dtype=np.float32)
        degr[0, :NPC] = deg[base : base + NPC]
        per_core[c]["degrow"] = degr
    return True, per_core


def _pack_core(T, tile_cols, Ttot, pc):
    """Pack one core's assignment into device arrays."""
    slots = Ttot * P
    idx16 = np.zeros((P, slots // 16), dtype=np.int16)
    dstl = np.full((P, Ttot), -1.0, dtype=np.float32)
    # idx slot position depends on the per-(group, window) instruction slot
    # index; dstl position is per global tile.  Build instruction slot maps.
    NBv = T.shape[0]
    b0 = 0
    while b0 < NBv:
        blocks = list(range(b0, min(b0 + GROUP, NBv)))
        for w in range(NW):
            # instruction covers tiles of (b in blocks, w) in order
            inst_t0 = tile_cols[(blocks[0], w)]
            for b in blocks:
                idxs, dls = pc["asn"][(b, w)]
                t0 = tile_cols[(b, w)]
                nslot = int(T[b, w]) * P
                # block's slot range within the instruction
                s_base = (t0 - inst_t0) * P
                arr = np.zeros(nslot, dtype=np.int16)
                arr[: len(idxs)] = idxs
                darr = np.full(nslot, -1.0, dtype=np.float32)
                darr[: len(dls)] = dls
                # dstl: slot k (tile t0 + k//P, partition k%P)
                kk = np.arange(nslot)
                dstl[kk % P, t0 + kk // P] = darr
                # idx: instruction slot i = s_base + k; col base inst_t0*8
                ii = s_base + kk
                ci = inst_t0 * (P // 16)
                for k8 in range(8):
                    idx16[16 * k8 + ii % 16, ci + ii // 16] = arr
        b0 += GROUP
    return {
        "idx16": idx16,
        "dstl": dstl,
        "invdeg": pc["invdeg"],
        "degrow": pc["degrow"],
    }


def _build_program(sched):
    import concourse.tile as tile
    from concourse import bacc, mybir

    f32 = mybir.dt.float32
    i16 = mybir.dt.int16

    T = sched["T"]
    tile_cols = sched["tile_cols"]
    Ttot = sched["Ttot"]
    slots = Ttot * P

    nc = bacc.Bacc(
        "TRN2",
        target_bir_lowering=False,
        debug=False,
        enable_asserts=False,
        num_devices=NCORES,
    )

    x_d = nc.dram_tensor("x", [N, D], f32, kind="ExternalInput").ap()
    idx_d = nc.dram_tensor("idx16", [P, slots // 16], i16, kind="ExternalInput").ap()
    dstl_d = nc.dram_tensor("dstl", [P, Ttot], f32, kind="ExternalInput").ap()
    invd_d = nc.dram_tensor("invdeg", [P, NB], f32, kind="ExternalInput").ap()
    degr_d = nc.dram_tensor("degrow", [1, NPAD], f32, kind="ExternalInput").ap()
    wt_d = nc.dram_tensor("wt", [D, D], f32, kind="ExternalInput").ap()
    brow_d = nc.dram_tensor("brow", [1, D], f32, kind="ExternalInput").ap()
    iota_d = nc.dram_tensor("iota", [P, P], f32, kind="ExternalInput").ap()
    out_d = nc.dram_tensor("out", [NPAD, D], f32, kind="ExternalOutput").ap()

    groups = []
    b0 = 0
    while b0 < NB:
        groups.append(list(range(b0, min(b0 + GROUP, NB))))
        b0 += GROUP

    with tile.TileContext(nc) as tc:
        with (
            tc.tile_pool(name="const", bufs=1) as cpool,
            tc.tile_pool(name="g", bufs=2) as gpool,
            tc.tile_pool(name="oh", bufs=6) as ohpool,
            tc.tile_pool(name="aggt", bufs=4) as atpool,
            tc.tile_pool(name="stage", bufs=3) as stpool,
            tc.tile_pool(name="pag", bufs=4, space="PSUM") as pagpool,
            tc.tile_pool(name="pout", bufs=4, space="PSUM") as poutpool,
        ):
            idx_s = cpool.tile([P, slots // 16], i16)
            nc.sync.dma_start(idx_s[:], idx_d[:, :])
            dstl_s = cpool.tile([P, Ttot], f32)
            nc.sync.dma_start(dstl_s[:], dstl_d[:, :])
            invd_s = cpool.tile([P, NB], f32)
            nc.sync.dma_start(invd_s[:], invd_d[:, :])
            degr_s = cpool.tile([1, NPAD], f32)
            nc.sync.dma_start(degr_s[:], degr_d[:, :])
            wt_s = cpool.tile([D, D], f32)
            nc.sync.dma_start(wt_s[:], wt_d[:, :])
            brow_s = cpool.tile([1, D], f32)
            nc.sync.dma_start(brow_s[:], brow_d[:, :])
            iota_s = cpool.tile([P, P], f32)
            nc.sync.dma_start(iota_s[:], iota_d[:, :])

            for blocks in groups:
                g_t0 = tile_cols[(blocks[0], 0)]  # first tile of group
                Tg = sum(int(T[b, w]) for b in blocks for w in range(NW))
                gt = gpool.tile([P, Tg * D], f32, tag="G")
                for w in range(NW):
                    w_t0 = tile_cols[(blocks[0], w)]
                    Tw = sum(int(T[b, w]) for b in blocks)
                    if Tw == 0:
                        continue
                    nw = Tw * P
                    o0 = (w_t0 - g_t0) * D
                    out_view = gt[:, o0 : o0 + Tw * D].rearrange(
                        "p (t f) -> p t f", f=D
                    )
                    ci = w_t0 * (P // 16)
                    nc.gpsimd.dma_gather(
                        out_view,
                        x_d[WBASE[w] : WBASE[w] + WIN, :],
                        idx_s[:, ci : ci + nw // 16],
                        nw,
                        nw,
                        D,
                        single_packet=False,
                    )
                ng = len(blocks)
                stage = stpool.tile([P, ng * D], f32, tag="stage")
                for bi, b in enumerate(blocks):
                    tiles = []
                    for w in range(NW):
                        t0 = tile_cols[(b, w)]
                        tiles += list(range(t0, t0 + int(T[b, w])))
                    pag = pagpool.tile([P, P], f32, tag="pag")
                    for k, t in enumerate(tiles):
                        oh = ohpool.tile([P, P], f32, tag="oh")
                        nc.vector.tensor_scalar(
                            out=oh[:],
                            in0=iota_s[:],
                            scalar1=dstl_s[:, t : t + 1],
                            scalar2=None,
                            op0=mybir.AluOpType.is_equal,
                        )
                        o = (t - g_t0) * D
                        nc.tensor.matmul(
                            out=pag[:],
                            lhsT=gt[:, o : o + D],
                            rhs=oh[:],
                            start=(k == 0),
                            stop=(k == len(tiles) - 1),
                        )
                    aggts = atpool.tile([P, P], f32, tag="aggt")
                    nc.scalar.copy(aggts[:], pag[:])
                    pout = poutpool.tile([P, P], f32, tag="pout")
                    nc.tensor.matmul(
                        out=pout[:], lhsT=aggts[:], rhs=wt_s[:], start=True, stop=False
                    )
                    nc.tensor.matmul(
                        out=pout[:],
                        lhsT=degr_s[:, b * P : (b + 1) * P],
                        rhs=brow_s[:],
                        start=False,
                        stop=True,
                    )
                    nc.scalar.mul(
                        stage[:, bi * D : (bi + 1) * D],
                        pout[:],
                        invd_s[:, b : b + 1],
                    )
                r0 = blocks[0] * P
                dst_view = out_d[r0 : r0 + ng * P, :].rearrange(
                    "(t p) f -> p t f", p=P
                )
                src_view = stage[:].rearrange("p (t f) -> p t f", f=D)
                nc.sync.dma_start(dst_view, src_view)

    nc.compile()
    return nc


_CACHED = None


def _get_program(sched):
    global _CACHED
    key = sched["T"].tobytes()
    if _CACHED is not None and _CACHED[0] == key:
        return _CACHED[1]
    nc = _build_program(sched)
    _CACHED = (key, nc)
    return nc


LAST_RESULTS = None


def kernel(x, edge_index, W, b, _trace=False):
    global LAST_RESULTS
    from concourse.bass_utils import run_bass_kernel_spmd

    x = np.ascontiguousarray(np.asarray(x, dtype=np.float32))
    W = np.asarray(W, dtype=np.float32)
    b = np.asarray(b, dtype=np.float32)

    sched, packed = _build_schedule(edge_index)
    nc = _get_program(sched)

    wt = np.ascontiguousarray(W.T).astype(np.float32)
    brow = b.reshape(1, D).astype(np.float32)
    iota = np.tile(np.arange(P, dtype=np.float32), (P, 1))

    in_maps = []
    for c in range(NCORES):
        m = dict(packed[c])
        m["x"] = x
        m["wt"] = wt
        m["brow"] = brow
        m["iota"] = iota
        in_maps.append(m)

    res = run_bass_kernel_spmd(
        nc, in_maps, core_ids=list(range(NCORES)), trace=_trace
    )
    LAST_RESULTS = res
    out = np.concatenate([res.results[c]["out"][:NPC] for c in range(NCORES)], axis=0)
    return out.astype(np.float32)



# revision 10
# speedup vs baseline: 2.0273x; 2.0273x over previous
"""GNN mean-aggregation conv kernel for Trainium2, 8-core SPMD.

Computes out[v] = (1/deg[v]) * sum_{(s,v) in E} (x[s] @ W.T + b), deg by dst.

Strategy: shard destination nodes across 8 cores (12500 rows each).  Use the
linearity of the op to aggregate raw x first and apply the 128x128 linear
second: out = (D^-1 A x) W^T + b*mask.  Edges are grouped by 128-dst block on
the host; each core gathers x[src] rows with dma_gather (int16 indices into
four overlapping 32768-row source windows), segment-sums them with one-hot
matmuls on the PE (aggT[f,d] += G[e,f]^T onehot[e,d]), then applies W^T, a
rank-1 deg*b term and a per-partition 1/deg scale:
out[d,j] = (sum_f aggT[f,d] Wt[f,j] + deg[d] b[j]) * inv_deg[d].
"""

import numpy as np

N, E, D = 100000, 640000, 128
NCORES = 8
NPC = N // NCORES            # dst nodes per core
P = 128                      # partition dim / dst block size
NB = (NPC + P - 1) // P      # 98 dst blocks per core
NPAD = NB * P                # 12544 padded dst rows per core
GROUP = 8                    # dst blocks per gather group
WIN = 32768                  # int16-addressable window
WBASE = [0, 22411, 44822, 67232]
NW = 4


def _build_schedule(edge_index):
    """Host-side prep.

    Returns (sched, per_core) where sched holds the shared tile structure
    (T[b][w] tile counts) and per_core the packed idx/dstl/deg arrays.
    """
    src = np.asarray(edge_index[0], dtype=np.int64)
    dst = np.asarray(edge_index[1], dtype=np.int64)

    deg = np.bincount(dst, minlength=N).astype(np.float32)
    inv_deg = np.where(deg > 0, 1.0 / np.maximum(deg, 1), 0.0).astype(np.float32)

    core = dst // NPC
    local = dst - core * NPC
    blk = local // P
    dstl = (local - blk * P).astype(np.float32)  # packed to bf16 in _pack_core

    # sort edges by (core, block, src)
    key = (core * NB + blk) * (N + 1) + src
    order = np.argsort(key, kind="stable")
    src_s = src[order]
    gblk_s = (core * NB + blk)[order]
    dstl_s = dstl[order]

    starts = np.searchsorted(gblk_s, np.arange(NCORES * NB + 1) - 0.5)

    # per (core, block): edge src arrays (sorted)
    def block_srcs(c, b):
        g = c * NB + b
        return src_s[starts[g] : starts[g + 1]], dstl_s[starts[g] : starts[g + 1]]

    # --- shared per-block window tile counts T[b][w] ---
    T = np.zeros((NB, NW), dtype=np.int64)
    for b in range(NB):
        # forward cumulative: edges that must be in windows <= w
        F = np.zeros(NW, dtype=np.int64)
        maxtot = 0
        for w in range(NW):
            hi = WBASE[w + 1] if w + 1 < NW else N
            m = 0
            for c in range(NCORES):
                s, _ = block_srcs(c, b)
                m = max(m, int(np.searchsorted(s, hi)))
            F[w] = (m + P - 1) // P
        for c in range(NCORES):
            s, _ = block_srcs(c, b)
            maxtot = max(maxtot, len(s))
        F[NW - 1] = max(F[NW - 1], (maxtot + P - 1) // P, 1)
        for w in range(1, NW):
            F[w] = max(F[w], F[w - 1])
        Tb = np.diff(np.concatenate([[0], F]))
        # backward: edges with src >= WBASE[w] must fit in suffix
        for w in range(NW - 1, 0, -1):
            m = 0
            for c in range(NCORES):
                s, _ = block_srcs(c, b)
                m = max(m, len(s) - int(np.searchsorted(s, WBASE[w])))
            need = (m + P - 1) // P
            while Tb[w:].sum() < need:
                Tb[w] += 1
        T[b] = Tb

    # --- per-core greedy assignment + packing, with retry on infeasibility ---
    for _attempt in range(20):
        ok, per_core = _try_pack(T, block_srcs, deg, inv_deg)
        if ok:
            break
        # _try_pack bumped T in place on failure
    else:
        raise RuntimeError("window assignment failed to converge")

    col_off = np.zeros(NB + 1, dtype=np.int64)  # global tile offset per block
    # global tile order: groups of GROUP blocks; within group: w-major, then b
    tile_cols = {}  # (b, w) -> first global tile col
    tcol = 0
    b0 = 0
    while b0 < NB:
        blocks = list(range(b0, min(b0 + GROUP, NB)))
        for w in range(NW):
            for b in blocks:
                tile_cols[(b, w)] = tcol
                tcol += int(T[b, w])
        b0 += GROUP
    Ttot = tcol

    sched = {"T": T, "tile_cols": tile_cols, "Ttot": Ttot}
    # repack per-core arrays into the global layout
    packed = [_pack_core(T, tile_cols, Ttot, pc) for pc in per_core]
    return sched, packed


def _try_pack(T, block_srcs, deg, inv_deg):
    """Greedy per-core window assignment. Returns (ok, per_core_raw).
    On infeasibility bumps T in place and returns (False, None)."""
    per_core = []
    for c in range(NCORES):
        core_asn = {}  # (b, w) -> (idx_list, dstl_list)
        for b in range(T.shape[0]):
            s, dl = block_srcs(c, b)
            n = len(s)
            used = np.zeros(n, dtype=bool)
            for w in range(NW):
                lo = WBASE[w]
                hi = lo + WIN
                cap = int(T[b, w]) * P
                # must-take: not yet used, src in window, and not eligible later
                nxt = WBASE[w + 1] if w + 1 < NW else N
                elig = (~used) & (s >= lo) & (s < hi)
                must = elig & (s < nxt)
                i_must = np.where(must)[0]
                if len(i_must) > cap:
                    T[b, w] += 1
                    return False, None
                take = list(i_must)
                i_opt = np.where(elig & ~must)[0]
                room = cap - len(take)
                take += list(i_opt[:room])
                used[take] = True
                core_asn[(b, w)] = (
                    (s[take] - lo).astype(np.int16),
                    dl[take].astype(np.float32),
                )
            if not used.all():
                T[b, NW - 1] += 1
                return False, None
        per_core.append({"asn": core_asn, "core": c})
    # attach deg data
    from ml_dtypes import bfloat16

    for c in range(NCORES):
        base = c * NPC
        tmp = np.zeros(NPAD, dtype=np.float32)
        tmp[:NPC] = inv_deg[base : base + NPC]
        per_core[c]["invdeg"] = np.ascontiguousarray(tmp.reshape(NB, P).T)
        degr = np.zeros((1, NPAD), dtype=np.float32)
        degr[0, :NPC] = deg[base : base + NPC]
        per_core[c]["degrow"] = degr.astype(bfloat16)  # deg < 256: exact in bf16
    return True, per_core


def _pack_core(T, tile_cols, Ttot, pc):
    """Pack one core's assignment into device arrays."""
    slots = Ttot * P
    from ml_dtypes import bfloat16

    idx16 = np.zeros((P, slots // 16), dtype=np.int16)
    dstl = np.full((P, Ttot), -1.0, dtype=np.float32)
    # idx slot position depends on the per-(group, window) instruction slot
    # index; dstl position is per global tile.  Build instruction slot maps.
    NBv = T.shape[0]
    b0 = 0
    while b0 < NBv:
        blocks = list(range(b0, min(b0 + GROUP, NBv)))
        for w in range(NW):
            # instruction covers tiles of (b in blocks, w) in order
            inst_t0 = tile_cols[(blocks[0], w)]
            for b in blocks:
                idxs, dls = pc["asn"][(b, w)]
                t0 = tile_cols[(b, w)]
                nslot = int(T[b, w]) * P
                # block's slot range within the instruction
                s_base = (t0 - inst_t0) * P
                arr = np.zeros(nslot, dtype=np.int16)
                arr[: len(idxs)] = idxs
                darr = np.full(nslot, -1.0, dtype=np.float32)
                darr[: len(dls)] = dls
                # dstl: slot k (tile t0 + k//P, partition k%P)
                kk = np.arange(nslot)
                dstl[kk % P, t0 + kk // P] = darr
                # idx: instruction slot i = s_base + k; col base inst_t0*8
                ii = s_base + kk
                ci = inst_t0 * (P // 16)
                for k8 in range(8):
                    idx16[16 * k8 + ii % 16, ci + ii // 16] = arr
        b0 += GROUP
    return {
        "idx16": idx16,
        "dstl": dstl.astype(bfloat16),  # values in {-1, 0..127}: exact in bf16
        "invdeg": pc["invdeg"],
        "degrow": pc["degrow"],
    }


def _build_program(sched):
    import concourse.tile as tile
    from concourse import bacc, mybir

    f32 = mybir.dt.float32
    bf16 = mybir.dt.bfloat16
    i16 = mybir.dt.int16

    T = sched["T"]
    tile_cols = sched["tile_cols"]
    Ttot = sched["Ttot"]
    slots = Ttot * P

    nc = bacc.Bacc(
        "TRN2",
        target_bir_lowering=False,
        debug=False,
        enable_asserts=False,
        num_devices=NCORES,
        num_swdge_queues=4,
    )

    x_d = nc.dram_tensor("x", [N, D], f32, kind="ExternalInput").ap()
    idx_d = nc.dram_tensor("idx16", [P, slots // 16], i16, kind="ExternalInput").ap()
    dstl_d = nc.dram_tensor("dstl", [P, Ttot], bf16, kind="ExternalInput").ap()
    invd_d = nc.dram_tensor("invdeg", [P, NB], f32, kind="ExternalInput").ap()
    degr_d = nc.dram_tensor("degrow", [1, NPAD], bf16, kind="ExternalInput").ap()
    wt_d = nc.dram_tensor("wt", [D, D], bf16, kind="ExternalInput").ap()
    brow_d = nc.dram_tensor("brow", [1, D], bf16, kind="ExternalInput").ap()
    iota_d = nc.dram_tensor("iota", [P, P], bf16, kind="ExternalInput").ap()
    out_d = nc.dram_tensor("out", [NPAD, D], f32, kind="ExternalOutput").ap()

    groups = []
    b0 = 0
    while b0 < NB:
        groups.append(list(range(b0, min(b0 + GROUP, NB))))
        b0 += GROUP

    with tile.TileContext(nc) as tc:
        with (
            tc.tile_pool(name="const", bufs=1) as cpool,
            tc.tile_pool(name="g", bufs=2) as gpool,
            tc.tile_pool(name="g16", bufs=2) as g16pool,
            tc.tile_pool(name="oh", bufs=2) as ohpool,
            tc.tile_pool(name="aggt", bufs=4) as atpool,
            tc.tile_pool(name="stage", bufs=3) as stpool,
            tc.tile_pool(name="pag", bufs=4, space="PSUM") as pagpool,
            tc.tile_pool(name="pout", bufs=4, space="PSUM") as poutpool,
        ):
            idx_s = cpool.tile([P, slots // 16], i16)
            nc.sync.dma_start(idx_s[:], idx_d[:, :])
            dstl_s = cpool.tile([P, Ttot], bf16)
            nc.sync.dma_start(dstl_s[:], dstl_d[:, :])
            invd_s = cpool.tile([P, NB], f32)
            nc.sync.dma_start(invd_s[:], invd_d[:, :])
            degr_s = cpool.tile([1, NPAD], bf16)
            nc.sync.dma_start(degr_s[:], degr_d[:, :])
            wt_s = cpool.tile([D, D], bf16)
            nc.sync.dma_start(wt_s[:], wt_d[:, :])
            brow_s = cpool.tile([1, D], bf16)
            nc.sync.dma_start(brow_s[:], brow_d[:, :])
            iota_s = cpool.tile([P, P], bf16)
            nc.sync.dma_start(iota_s[:], iota_d[:, :])

            for blocks in groups:
                g_t0 = tile_cols[(blocks[0], 0)]  # first tile of group
                Tg = sum(int(T[b, w]) for b in blocks for w in range(NW))
                gt = gpool.tile([P, Tg * D], f32, tag="G")
                for w in range(NW):
                    w_t0 = tile_cols[(blocks[0], w)]
                    Tw = sum(int(T[b, w]) for b in blocks)
                    if Tw == 0:
                        continue
                    nw = Tw * P
                    o0 = (w_t0 - g_t0) * D
                    out_view = gt[:, o0 : o0 + Tw * D].rearrange(
                        "p (t f) -> p t f", f=D
                    )
                    ci = w_t0 * (P // 16)
                    # one SWDGE queue per window: queue q activates Q7 cpu
                    # pair (2q, 2q+1), so the 4 window gathers' descriptor
                    # generation runs on all 8 Q7 cores concurrently
                    nc.gpsimd.dma_gather(
                        out_view,
                        x_d[WBASE[w] : WBASE[w] + WIN, :],
                        idx_s[:, ci : ci + nw // 16],
                        nw,
                        nw,
                        D,
                        single_packet=False,
                        queue_num=w,
                    )
                # f32 -> bf16 for fast PE weight loads (DVE)
                gt16 = g16pool.tile([P, Tg * D], bf16, tag="G16")
                nc.vector.tensor_copy(gt16[:], gt[:])
                # batched one-hot: oh[p, t, j] = (j == dstl[p, t]), bf16
                oh = ohpool.tile([P, Tg * D], bf16, tag="oh")
                nc.vector.tensor_tensor(
                    out=oh[:].rearrange("p (t f) -> p t f", f=D),
                    in0=iota_s[:].unsqueeze(1).broadcast_to((P, Tg, P)),
                    in1=dstl_s[:, g_t0 : g_t0 + Tg]
                    .unsqueeze(2)
                    .broadcast_to((P, Tg, P)),
                    op=mybir.AluOpType.is_equal,
                )
                ng = len(blocks)
                stage = stpool.tile([P, ng * D], f32, tag="stage")
                for bi, b in enumerate(blocks):
                    tiles = []
                    for w in range(NW):
                        t0 = tile_cols[(b, w)]
                        tiles += list(range(t0, t0 + int(T[b, w])))
                    pag = pagpool.tile([P, P], f32, tag="pag")
                    for k, t in enumerate(tiles):
                        o = (t - g_t0) * D
                        nc.tensor.matmul(
                            out=pag[:],
                            lhsT=gt16[:, o : o + D],
                            rhs=oh[:, o : o + D],
                            start=(k == 0),
                            stop=(k == len(tiles) - 1),
                        )
                    aggts = atpool.tile([P, P], bf16, tag="aggt")
                    nc.scalar.copy(aggts[:], pag[:])
                    pout = poutpool.tile([P, P], f32, tag="pout")
                    nc.tensor.matmul(
                        out=pout[:], lhsT=aggts[:], rhs=wt_s[:], start=True, stop=False
                    )
                    nc.tensor.matmul(
                        out=pout[:],
                        lhsT=degr_s[:, b * P : (b + 1) * P],
                        rhs=brow_s[:],
                        start=False,
                        stop=True,
                    )
                    nc.scalar.mul(
                        stage[:, bi * D : (bi + 1) * D],
                        pout[:],
                        invd_s[:, b : b + 1],
                    )
                r0 = blocks[0] * P
                dst_view = out_d[r0 : r0 + ng * P, :].rearrange(
                    "(t p) f -> p t f", p=P
                )
                src_view = stage[:].rearrange("p (t f) -> p t f", f=D)
                nc.sync.dma_start(dst_view, src_view)

    nc.compile()
    return nc


_CACHED = None


def _get_program(sched):
    global _CACHED
    key = sched["T"].tobytes()
    if _CACHED is not None and _CACHED[0] == key:
        return _CACHED[1]
    nc = _build_program(sched)
    _CACHED = (key, nc)
    return nc


LAST_RESULTS = None


def kernel(x, edge_index, W, b, _trace=False):
    global LAST_RESULTS
    from concourse.bass_utils import run_bass_kernel_spmd

    x = np.ascontiguousarray(np.asarray(x, dtype=np.float32))
    W = np.asarray(W, dtype=np.float32)
    b = np.asarray(b, dtype=np.float32)

    sched, packed = _build_schedule(edge_index)
    nc = _get_program(sched)

    from ml_dtypes import bfloat16

    wt = np.ascontiguousarray(W.T).astype(bfloat16)
    brow = b.reshape(1, D).astype(bfloat16)
    iota = np.tile(np.arange(P, dtype=np.float32), (P, 1)).astype(bfloat16)

    in_maps = []
    for c in range(NCORES):
        m = dict(packed[c])
        m["x"] = x
        m["wt"] = wt
        m["brow"] = brow
        m["iota"] = iota
        in_maps.append(m)

    res = run_bass_kernel_spmd(
        nc, in_maps, core_ids=list(range(NCORES)), trace=_trace
    )
    LAST_RESULTS = res
    out = np.concatenate([res.results[c]["out"][:NPC] for c in range(NCORES)], axis=0)
    return out.astype(np.float32)



# revision 13
# speedup vs baseline: 2.7767x; 1.3697x over previous
"""GNN mean-aggregation conv kernel for Trainium2, 8-core SPMD.

Computes out[v] = (1/deg[v]) * sum_{(s,v) in E} (x[s] @ W.T + b), deg by dst.

Strategy: shard destination nodes across 8 cores (12500 rows each).  Use the
linearity of the op to aggregate raw x first and apply the 128x128 linear
second: out = (D^-1 A x) W^T + b*mask.  Edges are grouped by 128-dst block on
the host; each core gathers x[src] rows with dma_gather (int16 indices into
four overlapping 32768-row source windows), segment-sums them with one-hot
matmuls on the PE (aggT[f,d] += G[e,f]^T onehot[e,d]), then applies W^T, a
rank-1 deg*b term and a per-partition 1/deg scale:
out[d,j] = (sum_f aggT[f,d] Wt[f,j] + deg[d] b[j]) * inv_deg[d].
"""

import numpy as np

N, E, D = 100000, 640000, 128
NCORES = 8
NPC = N // NCORES            # dst nodes per core
P = 128                      # partition dim / dst block size
NB = (NPC + P - 1) // P      # 98 dst blocks per core
NPAD = NB * P                # 12544 padded dst rows per core
GROUP = 8                    # dst blocks per gather group
WIN = 32768                  # int16-addressable window
# quartile window bases: equal must-take load per window = balanced SWDGE
# descriptor generation across the 4 Q7 queue pairs
WBASE = [0, 25000, 50000, 75000]
NW = 4


def _build_schedule(edge_index):
    """Host-side prep.

    Returns (sched, per_core) where sched holds the shared tile structure
    (T[b][w] tile counts) and per_core the packed idx/dstl/deg arrays.
    """
    src = np.asarray(edge_index[0], dtype=np.int64)
    dst = np.asarray(edge_index[1], dtype=np.int64)

    deg = np.bincount(dst, minlength=N).astype(np.float32)
    inv_deg = np.where(deg > 0, 1.0 / np.maximum(deg, 1), 0.0).astype(np.float32)

    core = dst // NPC
    local = dst - core * NPC
    blk = local // P
    dstl = (local - blk * P).astype(np.float32)  # packed to bf16 in _pack_core

    # sort edges by (core, block, src)
    key = (core * NB + blk) * (N + 1) + src
    order = np.argsort(key, kind="stable")
    src_s = src[order]
    gblk_s = (core * NB + blk)[order]
    dstl_s = dstl[order]

    starts = np.searchsorted(gblk_s, np.arange(NCORES * NB + 1) - 0.5)

    # per (core, block): edge src arrays (sorted)
    def block_srcs(c, b):
        g = c * NB + b
        return src_s[starts[g] : starts[g + 1]], dstl_s[starts[g] : starts[g + 1]]

    # --- shared per-block window tile counts T[b][w] ---
    T = np.zeros((NB, NW), dtype=np.int64)
    for b in range(NB):
        # forward cumulative: edges that must be in windows <= w
        F = np.zeros(NW, dtype=np.int64)
        maxtot = 0
        for w in range(NW):
            hi = WBASE[w + 1] if w + 1 < NW else N
            m = 0
            for c in range(NCORES):
                s, _ = block_srcs(c, b)
                m = max(m, int(np.searchsorted(s, hi)))
            F[w] = (m + P - 1) // P
        for c in range(NCORES):
            s, _ = block_srcs(c, b)
            maxtot = max(maxtot, len(s))
        F[NW - 1] = max(F[NW - 1], (maxtot + P - 1) // P, 1)
        for w in range(1, NW):
            F[w] = max(F[w], F[w - 1])
        Tb = np.diff(np.concatenate([[0], F]))
        # backward: edges with src >= WBASE[w] must fit in suffix
        for w in range(NW - 1, 0, -1):
            m = 0
            for c in range(NCORES):
                s, _ = block_srcs(c, b)
                m = max(m, len(s) - int(np.searchsorted(s, WBASE[w])))
            need = (m + P - 1) // P
            while Tb[w:].sum() < need:
                Tb[w] += 1
        T[b] = Tb

    # --- per-core greedy assignment + packing, with retry on infeasibility ---
    for _attempt in range(20):
        ok, per_core = _try_pack(T, block_srcs, deg, inv_deg)
        if ok:
            break
        # _try_pack bumped T in place on failure
    else:
        raise RuntimeError("window assignment failed to converge")

    col_off = np.zeros(NB + 1, dtype=np.int64)  # global tile offset per block
    # global tile order: groups of GROUP blocks; within group: w-major, then b
    tile_cols = {}  # (b, w) -> first global tile col
    tcol = 0
    b0 = 0
    while b0 < NB:
        blocks = list(range(b0, min(b0 + GROUP, NB)))
        for w in range(NW):
            for b in blocks:
                tile_cols[(b, w)] = tcol
                tcol += int(T[b, w])
        b0 += GROUP
    Ttot = tcol

    sched = {"T": T, "tile_cols": tile_cols, "Ttot": Ttot}
    # repack per-core arrays into the global layout
    packed = [_pack_core(T, tile_cols, Ttot, pc) for pc in per_core]
    return sched, packed


def _try_pack(T, block_srcs, deg, inv_deg):
    """Greedy per-core window assignment. Returns (ok, per_core_raw).
    On infeasibility bumps T in place and returns (False, None)."""
    per_core = []
    for c in range(NCORES):
        core_asn = {}  # (b, w) -> (idx_list, dstl_list)
        for b in range(T.shape[0]):
            s, dl = block_srcs(c, b)
            n = len(s)
            used = np.zeros(n, dtype=bool)
            for w in range(NW):
                lo = WBASE[w]
                hi = lo + WIN
                cap = int(T[b, w]) * P
                # must-take: not yet used, src in window, and not eligible later
                nxt = WBASE[w + 1] if w + 1 < NW else N
                elig = (~used) & (s >= lo) & (s < hi)
                must = elig & (s < nxt)
                i_must = np.where(must)[0]
                if len(i_must) > cap:
                    T[b, w] += 1
                    return False, None
                take = list(i_must)
                i_opt = np.where(elig & ~must)[0]
                room = cap - len(take)
                take += list(i_opt[:room])
                used[take] = True
                core_asn[(b, w)] = (
                    (s[take] - lo).astype(np.int16),
                    dl[take].astype(np.float32),
                )
            if not used.all():
                T[b, NW - 1] += 1
                return False, None
        per_core.append({"asn": core_asn, "core": c})
    # attach deg data
    from ml_dtypes import bfloat16

    for c in range(NCORES):
        base = c * NPC
        tmp = np.zeros(NPAD, dtype=np.float32)
        tmp[:NPC] = inv_deg[base : base + NPC]
        per_core[c]["invdeg"] = np.ascontiguousarray(tmp.reshape(NB, P).T)
        degr = np.zeros((1, NPAD), dtype=np.float32)
        degr[0, :NPC] = deg[base : base + NPC]
        per_core[c]["degrow"] = degr.astype(bfloat16)  # deg < 256: exact in bf16
    return True, per_core


def _pack_core(T, tile_cols, Ttot, pc):
    """Pack one core's assignment into device arrays."""
    slots = Ttot * P
    from ml_dtypes import bfloat16

    idx16 = np.zeros((P, slots // 16), dtype=np.int16)
    dstl = np.full((P, Ttot), -1.0, dtype=np.float32)
    # idx slot position depends on the per-(group, window) instruction slot
    # index; dstl position is per global tile.  Build instruction slot maps.
    NBv = T.shape[0]
    b0 = 0
    while b0 < NBv:
        blocks = list(range(b0, min(b0 + GROUP, NBv)))
        for w in range(NW):
            # instruction covers tiles of (b in blocks, w) in order
            inst_t0 = tile_cols[(blocks[0], w)]
            for b in blocks:
                idxs, dls = pc["asn"][(b, w)]
                t0 = tile_cols[(b, w)]
                nslot = int(T[b, w]) * P
                # block's slot range within the instruction
                s_base = (t0 - inst_t0) * P
                arr = np.zeros(nslot, dtype=np.int16)
                arr[: len(idxs)] = idxs
                darr = np.full(nslot, -1.0, dtype=np.float32)
                darr[: len(dls)] = dls
                # dstl: slot k (tile t0 + k//P, partition k%P)
                kk = np.arange(nslot)
                dstl[kk % P, t0 + kk // P] = darr
                # idx: instruction slot i = s_base + k; col base inst_t0*8
                ii = s_base + kk
                ci = inst_t0 * (P // 16)
                for k8 in range(8):
                    idx16[16 * k8 + ii % 16, ci + ii // 16] = arr
        b0 += GROUP
    return {
        "idx16": idx16,
        "dstl": dstl.astype(bfloat16),  # values in {-1, 0..127}: exact in bf16
        "invdeg": pc["invdeg"],
        "degrow": pc["degrow"],
    }


def _build_program(sched):
    import concourse.tile as tile
    from concourse import bacc, mybir

    f32 = mybir.dt.float32
    bf16 = mybir.dt.bfloat16
    i16 = mybir.dt.int16

    T = sched["T"]
    tile_cols = sched["tile_cols"]
    Ttot = sched["Ttot"]
    slots = Ttot * P

    nc = bacc.Bacc(
        "TRN2",
        target_bir_lowering=False,
        debug=False,
        enable_asserts=False,
        num_devices=NCORES,
        num_swdge_queues=4,
    )

    x_d = nc.dram_tensor("x", [N, D], f32, kind="ExternalInput").ap()
    idx_d = nc.dram_tensor("idx16", [P, slots // 16], i16, kind="ExternalInput").ap()
    dstl_d = nc.dram_tensor("dstl", [P, Ttot], bf16, kind="ExternalInput").ap()
    invd_d = nc.dram_tensor("invdeg", [P, NB], f32, kind="ExternalInput").ap()
    degr_d = nc.dram_tensor("degrow", [1, NPAD], bf16, kind="ExternalInput").ap()
    wt_d = nc.dram_tensor("wt", [D, D], bf16, kind="ExternalInput").ap()
    brow_d = nc.dram_tensor("brow", [1, D], bf16, kind="ExternalInput").ap()
    iota_d = nc.dram_tensor("iota", [P, P], bf16, kind="ExternalInput").ap()
    out_d = nc.dram_tensor("out", [NPAD, D], f32, kind="ExternalOutput").ap()

    groups = []
    b0 = 0
    while b0 < NB:
        groups.append(list(range(b0, min(b0 + GROUP, NB))))
        b0 += GROUP

    with tile.TileContext(nc) as tc:
        with (
            tc.tile_pool(name="const", bufs=1) as cpool,
            tc.tile_pool(name="g", bufs=2) as gpool,
            tc.tile_pool(name="g16", bufs=2) as g16pool,
            tc.tile_pool(name="oh", bufs=2) as ohpool,
            tc.tile_pool(name="aggt", bufs=4) as atpool,
            tc.tile_pool(name="stage", bufs=3) as stpool,
            tc.tile_pool(name="pag", bufs=4, space="PSUM") as pagpool,
            tc.tile_pool(name="pout", bufs=4, space="PSUM") as poutpool,
        ):
            idx_s = cpool.tile([P, slots // 16], i16)
            nc.sync.dma_start(idx_s[:], idx_d[:, :])
            dstl_s = cpool.tile([P, Ttot], bf16)
            nc.sync.dma_start(dstl_s[:], dstl_d[:, :])
            invd_s = cpool.tile([P, NB], f32)
            nc.sync.dma_start(invd_s[:], invd_d[:, :])
            degr_s = cpool.tile([1, NPAD], bf16)
            nc.sync.dma_start(degr_s[:], degr_d[:, :])
            wt_s = cpool.tile([D, D], bf16)
            nc.sync.dma_start(wt_s[:], wt_d[:, :])
            brow_s = cpool.tile([1, D], bf16)
            nc.sync.dma_start(brow_s[:], brow_d[:, :])
            iota_s = cpool.tile([P, P], bf16)
            nc.sync.dma_start(iota_s[:], iota_d[:, :])

            for blocks in groups:
                g_t0 = tile_cols[(blocks[0], 0)]  # first tile of group
                Tg = sum(int(T[b, w]) for b in blocks for w in range(NW))
                gt = gpool.tile([P, Tg * D], f32, tag="G")
                for w in range(NW):
                    w_t0 = tile_cols[(blocks[0], w)]
                    Tw = sum(int(T[b, w]) for b in blocks)
                    if Tw == 0:
                        continue
                    nw = Tw * P
                    o0 = (w_t0 - g_t0) * D
                    out_view = gt[:, o0 : o0 + Tw * D].rearrange(
                        "p (t f) -> p t f", f=D
                    )
                    ci = w_t0 * (P // 16)
                    # one SWDGE queue per window: queue q activates Q7 cpu
                    # pair (2q, 2q+1), so the 4 window gathers' descriptor
                    # generation runs on all 8 Q7 cores concurrently
                    nc.gpsimd.dma_gather(
                        out_view,
                        x_d[WBASE[w] : min(WBASE[w] + WIN, N), :],
                        idx_s[:, ci : ci + nw // 16],
                        nw,
                        nw,
                        D,
                        single_packet=False,
                        queue_num=w,
                    )
                # f32 -> bf16 for fast PE weight loads (Activation engine;
                # DVE's CAST path measured 4 cyc/col — Act is otherwise idle)
                gt16 = g16pool.tile([P, Tg * D], bf16, tag="G16")
                nc.scalar.copy(gt16[:], gt[:])
                # batched one-hot: oh[p, t, j] = (j == dstl[p, t]), bf16
                oh = ohpool.tile([P, Tg * D], bf16, tag="oh")
                nc.vector.tensor_tensor(
                    out=oh[:].rearrange("p (t f) -> p t f", f=D),
                    in0=iota_s[:].unsqueeze(1).broadcast_to((P, Tg, P)),
                    in1=dstl_s[:, g_t0 : g_t0 + Tg]
                    .unsqueeze(2)
                    .broadcast_to((P, Tg, P)),
                    op=mybir.AluOpType.is_equal,
                )
                ng = len(blocks)
                stage = stpool.tile([P, ng * D], f32, tag="stage")
                for bi, b in enumerate(blocks):
                    tiles = []
                    for w in range(NW):
                        t0 = tile_cols[(b, w)]
                        tiles += list(range(t0, t0 + int(T[b, w])))
                    pag = pagpool.tile([P, P], f32, tag="pag")
                    for k, t in enumerate(tiles):
                        o = (t - g_t0) * D
                        nc.tensor.matmul(
                            out=pag[:],
                            lhsT=gt16[:, o : o + D],
                            rhs=oh[:, o : o + D],
                            start=(k == 0),
                            stop=(k == len(tiles) - 1),
                        )
                    aggts = atpool.tile([P, P], bf16, tag="aggt")
                    nc.scalar.copy(aggts[:], pag[:])
                    pout = poutpool.tile([P, P], f32, tag="pout")
                    nc.tensor.matmul(
                        out=pout[:], lhsT=aggts[:], rhs=wt_s[:], start=True, stop=False
                    )
                    nc.tensor.matmul(
                        out=pout[:],
                        lhsT=degr_s[:, b * P : (b + 1) * P],
                        rhs=brow_s[:],
                        start=False,
                        stop=True,
                    )
                    nc.scalar.mul(
                        stage[:, bi * D : (bi + 1) * D],
                        pout[:],
                        invd_s[:, b : b + 1],
                    )
                r0 = blocks[0] * P
                dst_view = out_d[r0 : r0 + ng * P, :].rearrange(
                    "(t p) f -> p t f", p=P
                )
                src_view = stage[:].rearrange("p (t f) -> p t f", f=D)
                nc.sync.dma_start(dst_view, src_view)

    nc.compile()
    return nc


_CACHED = None


def _get_program(sched):
    global _CACHED
    key = sched["T"].tobytes()
    if _CACHED is not None and _CACHED[0] == key:
        return _CACHED[1]
    nc = _build_program(sched)
    _CACHED = (key, nc)
    return nc


LAST_RESULTS = None


def kernel(x, edge_index, W, b, _trace=False):
    global LAST_RESULTS
    from concourse.bass_utils import run_bass_kernel_spmd

    x = np.ascontiguousarray(np.asarray(x, dtype=np.float32))
    W = np.asarray(W, dtype=np.float32)
    b = np.asarray(b, dtype=np.float32)

    sched, packed = _build_schedule(edge_index)
    nc = _get_program(sched)

    from ml_dtypes import bfloat16

    wt = np.ascontiguousarray(W.T).astype(bfloat16)
    brow = b.reshape(1, D).astype(bfloat16)
    iota = np.tile(np.arange(P, dtype=np.float32), (P, 1)).astype(bfloat16)

    in_maps = []
    for c in range(NCORES):
        m = dict(packed[c])
        m["x"] = x
        m["wt"] = wt
        m["brow"] = brow
        m["iota"] = iota
        in_maps.append(m)

    res = run_bass_kernel_spmd(
        nc, in_maps, core_ids=list(range(NCORES)), trace=_trace
    )
    LAST_RESULTS = res
    out = np.concatenate([res.results[c]["out"][:NPC] for c in range(NCORES)], axis=0)
    return out.astype(np.float32)

